# revision 26
# baseline (speedup 1.0000x reference)
"""Distributed brute-force KNN (retrieval) kernel for 8 Trainium2 NeuronCores.

Strategy
--------
Candidates are sharded row-wise across the 8 cores (125k each). Each core
computes quantized scores for all 512 queries against its shard with the
tensor engine (bf16, exact integer-grid arithmetic), with two tricks folded
into augmented contraction rows:

  * per-query threshold rows: the matmul directly produces s~ - t_q, so a
    candidate "survives" iff its PSUM value is positive. Thresholds are
    picked host-side from a random sample so that ~50 candidates per core
    per query survive (a guaranteed superset of the global top-k).
  * index-embedding rows: a tiny per-column offset u = iw * 2^-18
    (iw = position within the 2048-wide window) is added. Because all
    positive quantities are exact multiples of 2^-18 below 2^5, the fp32
    PSUM value carries BOTH the score and the 11 index bits exactly.

A single DVE max8 pass per PSUM window then extracts the top-8 (value+index
packed) per query per window — no separate index pass. The host decodes the
embedded indices, rescores the ~400 surviving candidates per query exactly,
and merges to the global top-k.
"""

import numpy as np
import ml_dtypes

B, D, N = 512, 64, 1_000_000
NCORES = 8
NSHARD = N // NCORES            # 125000
WIN = 2048                      # candidates per max8 window (4 PSUM banks)
NWIN = (NSHARD + WIN - 1) // WIN  # 62
NPAD = NWIN * WIN               # 126976
KAUG = 68                       # 64 dims + t_hi + t_lo + u_hi + u_lo
QB = B // 128                   # 4 query blocks

SAMP = 32768                    # host-side sample size per core
RSTAR = 14                      # threshold = (RSTAR-th largest sample) - 1/128

_Q_GRID = 8.0                   # queries quantized to 1/8
_C_GRID = 16.0                  # candidates quantized to 1/16  -> score grid 1/128
_EMB = 2.0 ** 18                # index embedding unit 2^-18 (11 bits per window)

# Window split between the two PSUM-draining engines, balanced to their
# per-element rates (DVE max8 ~1.10ns/el vs ACT relu+accum ~0.93ns/el),
# evenly interleaved so both engines stay busy throughout.
def _window_split(nwin):
    nd = max(1, round(nwin * 31 / 62))
    dv = [w for w in range(nwin) if (w + 1) * nd // nwin > w * nd // nwin]
    ac = [w for w in range(nwin) if w not in set(dv)]
    return (dv, ac,
            {w: i for i, w in enumerate(dv)},
            {w: i for i, w in enumerate(ac)})


DVE_WINS, ACT_WINS, _DVE_RANK, _ACT_RANK = _window_split(NWIN)

_CACHE = {}


def _build_bass():
    import concourse.tile as tile
    import concourse.mybir as mybir
    from concourse import bacc

    nc = bacc.Bacc("TRN2", target_bir_lowering=False, debug=False,
                   enable_asserts=False)
    q_dram = nc.dram_tensor("qaug", (KAUG, B), mybir.dt.bfloat16,
                            kind="ExternalInput")
    c_dram = nc.dram_tensor("caug", (KAUG, NPAD), mybir.dt.bfloat16,
                            kind="ExternalInput")
    NDVE = len(DVE_WINS)            # DVE max8 windows (2 halves each)
    NACT = len(ACT_WINS)            # ACT relu+accum windows (2 halves each)
    HW_ = WIN // 2                  # 1024: one [128, 1024] PSUM tile = 2 banks
    out_dram = nc.dram_tensor("out_vals", (B, NDVE * 16), mybir.dt.float32,
                              kind="ExternalOutput")
    acc_dram = nc.dram_tensor("out_acc", (B, NACT * 2), mybir.dt.float32,
                              kind="ExternalOutput")

    with tile.TileContext(nc) as tc:
        with tc.tile_pool(name="cts", bufs=3) as ctp, \
             tc.tile_pool(name="qp", bufs=1) as qp, \
             tc.tile_pool(name="outp", bufs=1) as outp, \
             tc.tile_pool(name="aop", bufs=2) as aop, \
             tc.tile_pool(name="ps", bufs=1, space="PSUM") as psp:

            qt = qp.tile([KAUG, B], mybir.dt.bfloat16)
            nc.gpsimd.dma_start(qt[:], q_dram.ap()[:, :])

            ov = [outp.tile([128, NDVE * 16], mybir.dt.float32, tag=f"ov{qb}",
                            name=f"ov{qb}")
                  for qb in range(QB)]
            oa = [outp.tile([128, NACT * 2], mybir.dt.float32, tag=f"oa{qb}",
                            name=f"oa{qb}")
                  for qb in range(QB)]

            for w in range(NWIN):
                ct = ctp.tile([KAUG, WIN], mybir.dt.bfloat16, tag="ct")
                nc.gpsimd.dma_start(ct[:], c_dram.ap()[:, w * WIN:(w + 1) * WIN])
                eng = "D" if w in _DVE_RANK else "A"
                for qb in range(QB):
                    for h in range(2):
                        # separate PSUM tag sets per consumer engine so each
                        # WAR chain serializes on one engine only
                        pt = psp.tile([128, HW_], mybir.dt.float32,
                                      tag=f"ps{eng}{h}", name="pt")
                        for s in range(2):
                            col = h * HW_ + s * 512
                            nc.tensor.matmul(pt[:, s * 512:(s + 1) * 512],
                                             qt[:, qb * 128:(qb + 1) * 128],
                                             ct[:, col:col + 512],
                                             start=True, stop=True)
                        if eng == "D":
                            o = _DVE_RANK[w] * 16 + h * 8
                            nc.vector.max(ov[qb][:, o:o + 8], pt[:])
                        else:
                            ao = aop.tile([128, HW_], mybir.dt.float32,
                                          tag="ao", name="ao")
                            a = _ACT_RANK[w] * 2 + h
                            nc.scalar.activation(
                                ao[:], pt[:],
                                mybir.ActivationFunctionType.Relu,
                                accum_out=oa[qb][:, a:a + 1])

            for qb in range(QB):
                nc.gpsimd.dma_start(out_dram.ap()[qb * 128:(qb + 1) * 128, :],
                                    ov[qb][:])
                nc.gpsimd.dma_start(acc_dram.ap()[qb * 128:(qb + 1) * 128, :],
                                    oa[qb][:])
    nc.compile()
    return nc


def _get_nc():
    if "nc" not in _CACHE:
        _CACHE["nc"] = _build_bass()
    return _CACHE["nc"]


def _bf16(a):
    """Exact fp32->bf16 for values already representable in bf16 (bit shift —
    much faster than ml_dtypes astype; truncation == rounding here)."""
    return (np.ascontiguousarray(a, np.float32).view(np.uint32) >> 16) \
        .astype(np.uint16).view(ml_dtypes.bfloat16)


def _prep_inputs(queries, candidates):
    """Host-side staging: quantize, sample thresholds, build augmented operands."""
    qq = np.round(queries.astype(np.float32) * _Q_GRID) / _Q_GRID
    cc = np.round(candidates.astype(np.float32) * _C_GRID) / _C_GRID

    rng = np.random.default_rng(0x5EED)
    iw = np.arange(NPAD, dtype=np.int64) % WIN
    u_hi = ((iw >> 6).astype(np.float32)) * (2.0 ** -12)   # 5 bits, bf16-exact
    u_lo = ((iw & 63).astype(np.float32)) * (2.0 ** -18)   # 6 bits, bf16-exact

    in_maps = []
    t_all = np.zeros((NCORES, B), np.float32)
    for c in range(NCORES):
        shard = cc[c * NSHARD:(c + 1) * NSHARD]            # [125000, 64]
        sidx = rng.choice(NSHARD, SAMP, replace=False)
        s_samp = qq @ shard[sidx].T                        # [512, SAMP] exact fp32
        t_raw = np.partition(s_samp, SAMP - RSTAR, axis=1)[:, SAMP - RSTAR]
        t = (t_raw - np.float32(1.0 / 128.0)).astype(np.float32)
        t_all[c] = t                                       # on grid, strictly below
        t_hi = np.floor(t)
        t_lo = (t - t_hi).astype(np.float32)

        qaug = np.zeros((KAUG, B), np.float32)
        qaug[:D] = qq.T
        qaug[D] = -t_hi
        qaug[D + 1] = -t_lo
        qaug[D + 2] = 1.0
        qaug[D + 3] = 1.0

        caug = np.zeros((KAUG, NPAD), np.float32)
        caug[:D, :NSHARD] = shard.T
        caug[D] = 1.0
        caug[D + 1] = 1.0
        caug[D + 2] = u_hi
        caug[D + 3] = u_lo

        in_maps.append({"qaug": _bf16(qaug), "caug": _bf16(caug)})
    return in_maps, qq, cc, t_all


def _u_of(iw):
    """Exact fp32 embedding offset u(iw), matching the device aug rows."""
    return (((iw >> 6).astype(np.float32) * np.float32(2.0 ** -12))
            + (iw & 63).astype(np.float32) * np.float32(2.0 ** -18))


def _decode_and_merge(queries, candidates, core_outs, qq, cc, t_all, k):
    """Decode embedded indices, rescore survivors exactly, global top-k.

    Even windows come from DVE max8 (top-8 packed values per window). Odd
    windows come from ACT relu+accum: the accumulated value IS the packed
    survivor when the window held exactly one; the exact-fp32 verification
    below provably rejects every other case (any extra survivor shifts the
    sum by >= 1/128, any non-survivor decode misses by >= 1/128), and those
    windows are recovered by an exact host rescan on the quantized grid.
    """
    HW_ = WIN // 2
    dve_wins = np.array(DVE_WINS, np.int64)
    act_wins = np.array(ACT_WINS, np.int64)
    qn, cidx_all = [], []
    rescan = []                                            # (core, q, halfblock)
    for c, (ov, oa) in enumerate(core_outs):
        # --- DVE max8 windows, 16 slots per window (2 halves x 8) ---
        ov = np.asarray(ov, np.float32)
        qi, slot = np.nonzero(ov > 0)
        v = ov[qi, slot]
        m = np.rint(v.astype(np.float64) * _EMB).astype(np.int64)
        iw = m % WIN
        cand_local = dve_wins[slot // 16] * WIN + iw
        ok = cand_local < NSHARD
        qn.append(qi[ok])
        cidx_all.append(cand_local[ok] + c * NSHARD)
        # --- ACT accum half-windows (2 accum cols per window) ---
        oa = np.asarray(oa, np.float32)
        qi2, col = np.nonzero(oa > 0)
        a = oa[qi2, col]
        m2 = np.rint(a.astype(np.float64) * _EMB).astype(np.int64)
        iw2 = (m2 % WIN).astype(np.int64)
        gw = act_wins[col // 2]
        cand_local2 = gw * WIN + iw2
        inb = cand_local2 < NSHARD
        # exact verification: fp32((s~ - t) + u) must equal the accum bitwise
        vc = np.full(a.shape, np.float32(np.nan), np.float32)
        if inb.any():
            s_ex = np.einsum("md,md->m", qq[qi2[inb]],
                             cc[c * NSHARD + cand_local2[inb]],
                             dtype=np.float32, casting="no")
            vc[inb] = (s_ex - t_all[c, qi2[inb]]).astype(np.float32) \
                + _u_of(iw2[inb]).astype(np.float32)
        # the decoded candidate must also lie in the accumulated half-window
        half_ok = (iw2 // HW_) == (col % 2)
        good = inb & half_ok & (vc == a)
        qn.append(qi2[good])
        cidx_all.append(cand_local2[good] + c * NSHARD)
        bad = ~good
        for q, w, hf in zip(qi2[bad], gw[bad], (col % 2)[bad]):
            rescan.append((c, q, w * WIN + hf * HW_))
    # --- rescan unresolved ACT half-windows with exact grid arithmetic ---
    if rescan:
        from collections import defaultdict
        groups = defaultdict(list)
        for c, q, blk in rescan:
            groups[(c, blk)].append(q)
        for (c, blk), qs in groups.items():
            qs = np.array(qs)
            lo = c * NSHARD + blk
            hi = min(lo + HW_, (c + 1) * NSHARD)
            if hi <= lo:
                continue
            s_blk = qq[qs] @ cc[lo:hi].T                   # exact fp32 grid
            r, cnd = np.nonzero(s_blk > t_all[c, qs][:, None])
            qn.append(qs[r])
            cidx_all.append(lo + cnd)
    qi = np.concatenate(qn)
    ci = np.concatenate(cidx_all)

    # exact rescore of survivors in float64, then order like jax.lax.top_k
    qf = queries.astype(np.float64)
    cf = candidates.astype(np.float64)
    vals = np.einsum("md,md->m", qf[qi], cf[ci])
    vals32 = vals.astype(np.float32)

    order = np.lexsort((ci, -vals, qi))
    qi, ci, vals32 = qi[order], ci[order], vals32[order]
    counts = np.bincount(qi, minlength=B)

    out_v = np.zeros((B, k), np.float32)
    out_i = np.zeros((B, k), np.int32)
    starts = np.concatenate(([0], np.cumsum(counts)))
    for b in range(B):
        s, e = starts[b], starts[b + 1]
        if e - s < k:   # statistical fallback — should essentially never happen
            sc = queries[b].astype(np.float64) @ candidates.astype(np.float64).T
            top = np.argpartition(-sc, k)[:k]
            top = top[np.lexsort((top, -sc[top]))]
            out_v[b] = sc[top].astype(np.float32)
            out_i[b] = top.astype(np.int32)
            continue
        out_v[b] = vals32[s:s + k]
        out_i[b] = ci[s:s + k].astype(np.int32)
    return out_v, out_i


def kernel(queries, candidates, k):
    import os
    from concourse import bass_utils

    k = int(k)
    queries = np.asarray(queries, np.float32)
    candidates = np.asarray(candidates, np.float32)
    in_maps, qq, cc, t_all = _prep_inputs(queries, candidates)
    nc = _get_nc()
    trace = os.environ.get("KNN_TRACE", "0") == "1"
    try:
        res = bass_utils.run_bass_kernel_spmd(nc, in_maps,
                                              core_ids=list(range(NCORES)),
                                              trace=trace)
    except ModuleNotFoundError:
        res = bass_utils.run_bass_kernel_spmd(nc, in_maps,
                                              core_ids=list(range(NCORES)))
    _CACHE["last_results"] = res
    core_outs = [(r["out_vals"], r["out_acc"]) for r in res.results]
    return _decode_and_merge(queries, candidates, core_outs, qq, cc, t_all, k)


# revision 27
# speedup vs baseline: 1.0080x; 1.0080x over previous
"""Distributed brute-force KNN (retrieval) kernel for 8 Trainium2 NeuronCores.

Strategy
--------
Candidates are sharded row-wise across the 8 cores (125k each). Each core
computes quantized scores for all 512 queries against its shard with the
tensor engine (bf16, exact integer-grid arithmetic), with two tricks folded
into augmented contraction rows:

  * per-query threshold rows: the matmul directly produces s~ - t_q, so a
    candidate "survives" iff its PSUM value is positive. Thresholds are
    picked host-side from a random sample so that ~50 candidates per core
    per query survive (a guaranteed superset of the global top-k).
  * index-embedding rows: a tiny per-column offset u = iw * 2^-18
    (iw = position within the 2048-wide window) is added. Because all
    positive quantities are exact multiples of 2^-18 below 2^5, the fp32
    PSUM value carries BOTH the score and the 11 index bits exactly.

A single DVE max8 pass per PSUM window then extracts the top-8 (value+index
packed) per query per window — no separate index pass. The host decodes the
embedded indices, rescores the ~400 surviving candidates per query exactly,
and merges to the global top-k.
"""

import numpy as np
import ml_dtypes

B, D, N = 512, 64, 1_000_000
NCORES = 8
NSHARD = N // NCORES            # 125000
WIN = 2048                      # candidates per max8 window (4 PSUM banks)
NWIN = (NSHARD + WIN - 1) // WIN  # 62
NPAD = NWIN * WIN               # 126976
KAUG = 68                       # 64 dims + t_hi + t_lo + u_hi + u_lo
QB = B // 128                   # 4 query blocks

SAMP = 32768                    # host-side sample size per core
RSTAR = 14                      # threshold = (RSTAR-th largest sample) - 1/128

_Q_GRID = 8.0                   # queries quantized to 1/8
_C_GRID = 16.0                  # candidates quantized to 1/16  -> score grid 1/128
_EMB = 2.0 ** 18                # index embedding unit 2^-18 (11 bits per window)

# Window split between the two PSUM-draining engines, balanced to their
# per-element rates (DVE max8 ~1.10ns/el vs ACT relu+accum ~0.93ns/el),
# evenly interleaved so both engines stay busy throughout.
def _window_split(nwin):
    nd = max(1, round(nwin * 31 / 62))
    dv = [w for w in range(nwin) if (w + 1) * nd // nwin > w * nd // nwin]
    ac = [w for w in range(nwin) if w not in set(dv)]
    return (dv, ac,
            {w: i for i, w in enumerate(dv)},
            {w: i for i, w in enumerate(ac)})


DVE_WINS, ACT_WINS, _DVE_RANK, _ACT_RANK = _window_split(NWIN)

_CACHE = {}


def _build_bass():
    import concourse.tile as tile
    import concourse.mybir as mybir
    from concourse import bacc

    nc = bacc.Bacc("TRN2", target_bir_lowering=False, debug=False,
                   enable_asserts=False)
    q_dram = nc.dram_tensor("qaug", (KAUG, B), mybir.dt.bfloat16,
                            kind="ExternalInput")
    c_dram = nc.dram_tensor("caug", (KAUG, NPAD), mybir.dt.bfloat16,
                            kind="ExternalInput")
    NDVE = len(DVE_WINS)            # DVE max8 windows (2 halves each)
    NACT = len(ACT_WINS)            # ACT relu+accum windows (2 halves each)
    HW_ = WIN // 2                  # 1024: one [128, 1024] PSUM tile = 2 banks
    out_dram = nc.dram_tensor("out_vals", (B, NDVE * 16), mybir.dt.float32,
                              kind="ExternalOutput")
    acc_dram = nc.dram_tensor("out_acc", (B, NACT * 2), mybir.dt.float32,
                              kind="ExternalOutput")

    with tile.TileContext(nc) as tc:
        with tc.tile_pool(name="cts", bufs=6) as ctp, \
             tc.tile_pool(name="qp", bufs=1) as qp, \
             tc.tile_pool(name="outp", bufs=1) as outp, \
             tc.tile_pool(name="aop", bufs=4) as aop, \
             tc.tile_pool(name="ps", bufs=1, space="PSUM") as psp:

            qt = qp.tile([KAUG, B], mybir.dt.bfloat16)
            nc.gpsimd.dma_start(qt[:], q_dram.ap()[:, :])

            ov = [outp.tile([128, NDVE * 16], mybir.dt.float32, tag=f"ov{qb}",
                            name=f"ov{qb}")
                  for qb in range(QB)]
            oa = [outp.tile([128, NACT * 2], mybir.dt.float32, tag=f"oa{qb}",
                            name=f"oa{qb}")
                  for qb in range(QB)]

            for w in range(NWIN):
                ct = ctp.tile([KAUG, WIN], mybir.dt.bfloat16, tag="ct")
                nc.gpsimd.dma_start(ct[:], c_dram.ap()[:, w * WIN:(w + 1) * WIN])
                eng = "D" if w in _DVE_RANK else "A"
                for qb in range(QB):
                    for h in range(2):
                        # separate PSUM tag sets per consumer engine so each
                        # WAR chain serializes on one engine only
                        pt = psp.tile([128, HW_], mybir.dt.float32,
                                      tag=f"ps{eng}{h}", name="pt")
                        for s in range(2):
                            col = h * HW_ + s * 512
                            nc.tensor.matmul(pt[:, s * 512:(s + 1) * 512],
                                             qt[:, qb * 128:(qb + 1) * 128],
                                             ct[:, col:col + 512],
                                             start=True, stop=True)
                        if eng == "D":
                            o = _DVE_RANK[w] * 16 + h * 8
                            nc.vector.max(ov[qb][:, o:o + 8], pt[:])
                        else:
                            ao = aop.tile([128, HW_], mybir.dt.float32,
                                          tag="ao", name="ao")
                            a = _ACT_RANK[w] * 2 + h
                            nc.scalar.activation(
                                ao[:], pt[:],
                                mybir.ActivationFunctionType.Relu,
                                accum_out=oa[qb][:, a:a + 1])

            for qb in range(QB):
                nc.gpsimd.dma_start(out_dram.ap()[qb * 128:(qb + 1) * 128, :],
                                    ov[qb][:])
                nc.gpsimd.dma_start(acc_dram.ap()[qb * 128:(qb + 1) * 128, :],
                                    oa[qb][:])
    nc.compile()
    return nc


def _get_nc():
    if "nc" not in _CACHE:
        _CACHE["nc"] = _build_bass()
    return _CACHE["nc"]


def _bf16(a):
    """Exact fp32->bf16 for values already representable in bf16 (bit shift —
    much faster than ml_dtypes astype; truncation == rounding here)."""
    return (np.ascontiguousarray(a, np.float32).view(np.uint32) >> 16) \
        .astype(np.uint16).view(ml_dtypes.bfloat16)


def _prep_inputs(queries, candidates):
    """Host-side staging: quantize, sample thresholds, build augmented operands."""
    qq = np.round(queries.astype(np.float32) * _Q_GRID) / _Q_GRID
    cc = np.round(candidates.astype(np.float32) * _C_GRID) / _C_GRID

    rng = np.random.default_rng(0x5EED)
    iw = np.arange(NPAD, dtype=np.int64) % WIN
    u_hi = ((iw >> 6).astype(np.float32)) * (2.0 ** -12)   # 5 bits, bf16-exact
    u_lo = ((iw & 63).astype(np.float32)) * (2.0 ** -18)   # 6 bits, bf16-exact

    in_maps = []
    t_all = np.zeros((NCORES, B), np.float32)
    for c in range(NCORES):
        shard = cc[c * NSHARD:(c + 1) * NSHARD]            # [125000, 64]
        sidx = rng.choice(NSHARD, SAMP, replace=False)
        s_samp = qq @ shard[sidx].T                        # [512, SAMP] exact fp32
        t_raw = np.partition(s_samp, SAMP - RSTAR, axis=1)[:, SAMP - RSTAR]
        t = (t_raw - np.float32(1.0 / 128.0)).astype(np.float32)
        t_all[c] = t                                       # on grid, strictly below
        t_hi = np.floor(t)
        t_lo = (t - t_hi).astype(np.float32)

        qaug = np.zeros((KAUG, B), np.float32)
        qaug[:D] = qq.T
        qaug[D] = -t_hi
        qaug[D + 1] = -t_lo
        qaug[D + 2] = 1.0
        qaug[D + 3] = 1.0

        caug = np.zeros((KAUG, NPAD), np.float32)
        caug[:D, :NSHARD] = shard.T
        caug[D] = 1.0
        caug[D + 1] = 1.0
        caug[D + 2] = u_hi
        caug[D + 3] = u_lo

        in_maps.append({"qaug": _bf16(qaug), "caug": _bf16(caug)})
    return in_maps, qq, cc, t_all


def _u_of(iw):
    """Exact fp32 embedding offset u(iw), matching the device aug rows."""
    return (((iw >> 6).astype(np.float32) * np.float32(2.0 ** -12))
            + (iw & 63).astype(np.float32) * np.float32(2.0 ** -18))


def _decode_and_merge(queries, candidates, core_outs, qq, cc, t_all, k):
    """Decode embedded indices, rescore survivors exactly, global top-k.

    Even windows come from DVE max8 (top-8 packed values per window). Odd
    windows come from ACT relu+accum: the accumulated value IS the packed
    survivor when the window held exactly one; the exact-fp32 verification
    below provably rejects every other case (any extra survivor shifts the
    sum by >= 1/128, any non-survivor decode misses by >= 1/128), and those
    windows are recovered by an exact host rescan on the quantized grid.
    """
    HW_ = WIN // 2
    dve_wins = np.array(DVE_WINS, np.int64)
    act_wins = np.array(ACT_WINS, np.int64)
    qn, cidx_all = [], []
    rescan = []                                            # (core, q, halfblock)
    for c, (ov, oa) in enumerate(core_outs):
        # --- DVE max8 windows, 16 slots per window (2 halves x 8) ---
        ov = np.asarray(ov, np.float32)
        qi, slot = np.nonzero(ov > 0)
        v = ov[qi, slot]
        m = np.rint(v.astype(np.float64) * _EMB).astype(np.int64)
        iw = m % WIN
        cand_local = dve_wins[slot // 16] * WIN + iw
        ok = cand_local < NSHARD
        qn.append(qi[ok])
        cidx_all.append(cand_local[ok] + c * NSHARD)
        # --- ACT accum half-windows (2 accum cols per window) ---
        oa = np.asarray(oa, np.float32)
        qi2, col = np.nonzero(oa > 0)
        a = oa[qi2, col]
        m2 = np.rint(a.astype(np.float64) * _EMB).astype(np.int64)
        iw2 = (m2 % WIN).astype(np.int64)
        gw = act_wins[col // 2]
        cand_local2 = gw * WIN + iw2
        inb = cand_local2 < NSHARD
        # exact verification: fp32((s~ - t) + u) must equal the accum bitwise
        vc = np.full(a.shape, np.float32(np.nan), np.float32)
        if inb.any():
            s_ex = np.einsum("md,md->m", qq[qi2[inb]],
                             cc[c * NSHARD + cand_local2[inb]],
                             dtype=np.float32, casting="no")
            vc[inb] = (s_ex - t_all[c, qi2[inb]]).astype(np.float32) \
                + _u_of(iw2[inb]).astype(np.float32)
        # the decoded candidate must also lie in the accumulated half-window
        half_ok = (iw2 // HW_) == (col % 2)
        good = inb & half_ok & (vc == a)
        qn.append(qi2[good])
        cidx_all.append(cand_local2[good] + c * NSHARD)
        bad = ~good
        for q, w, hf in zip(qi2[bad], gw[bad], (col % 2)[bad]):
            rescan.append((c, q, w * WIN + hf * HW_))
    # --- rescan unresolved ACT half-windows with exact grid arithmetic ---
    if rescan:
        from collections import defaultdict
        groups = defaultdict(list)
        for c, q, blk in rescan:
            groups[(c, blk)].append(q)
        for (c, blk), qs in groups.items():
            qs = np.array(qs)
            lo = c * NSHARD + blk
            hi = min(lo + HW_, (c + 1) * NSHARD)
            if hi <= lo:
                continue
            s_blk = qq[qs] @ cc[lo:hi].T                   # exact fp32 grid
            r, cnd = np.nonzero(s_blk > t_all[c, qs][:, None])
            qn.append(qs[r])
            cidx_all.append(lo + cnd)
    qi = np.concatenate(qn)
    ci = np.concatenate(cidx_all)

    # exact rescore of survivors in float64, then order like jax.lax.top_k
    qf = queries.astype(np.float64)
    cf = candidates.astype(np.float64)
    vals = np.einsum("md,md->m", qf[qi], cf[ci])
    vals32 = vals.astype(np.float32)

    order = np.lexsort((ci, -vals, qi))
    qi, ci, vals32 = qi[order], ci[order], vals32[order]
    counts = np.bincount(qi, minlength=B)

    out_v = np.zeros((B, k), np.float32)
    out_i = np.zeros((B, k), np.int32)
    starts = np.concatenate(([0], np.cumsum(counts)))
    for b in range(B):
        s, e = starts[b], starts[b + 1]
        if e - s < k:   # statistical fallback — should essentially never happen
            sc = queries[b].astype(np.float64) @ candidates.astype(np.float64).T
            top = np.argpartition(-sc, k)[:k]
            top = top[np.lexsort((top, -sc[top]))]
            out_v[b] = sc[top].astype(np.float32)
            out_i[b] = top.astype(np.int32)
            continue
        out_v[b] = vals32[s:s + k]
        out_i[b] = ci[s:s + k].astype(np.int32)
    return out_v, out_i


def kernel(queries, candidates, k):
    import os
    from concourse import bass_utils

    k = int(k)
    queries = np.asarray(queries, np.float32)
    candidates = np.asarray(candidates, np.float32)
    in_maps, qq, cc, t_all = _prep_inputs(queries, candidates)
    nc = _get_nc()
    trace = os.environ.get("KNN_TRACE", "0") == "1"
    try:
        res = bass_utils.run_bass_kernel_spmd(nc, in_maps,
                                              core_ids=list(range(NCORES)),
                                              trace=trace)
    except ModuleNotFoundError:
        res = bass_utils.run_bass_kernel_spmd(nc, in_maps,
                                              core_ids=list(range(NCORES)))
    _CACHE["last_results"] = res
    core_outs = [(r["out_vals"], r["out_acc"]) for r in res.results]
    return _decode_and_merge(queries, candidates, core_outs, qq, cc, t_all, k)
